# revision 62
# baseline (speedup 1.0000x reference)
"""Distributed Trainium2 (Bass/Tile) kernel for single-head latent attention.

Reference computation (B=4, S=4096, D=1024, DL=64):
    qkv = x @ Wd + bd; q,k,v = split(qkv)
    logits = (q @ k^T) / sqrt(DL) / TEMP, key-masked
    out = softmax(logits) @ v @ Wu + bu

Sharding: data-parallel over (batch, seq-half) -> 8 shards of 2048 query
rows; each core recomputes K/V for its batch's compacted keys (no
collectives).

Design notes (vs the 124us baseline):
  - Host-side mask compaction (only ~2048 unmasked keys kept, cap 2176).
  - Inputs are pre-split host-side into C-contiguous per-transfer blocks and
    the SBUF side is laid out identically, so both ends of every DMA are
    4KB+/partition contiguous -> big packets -> full HW-queue bandwidth.
    Only sync + scalar have hardware DGE queues; gpsimd's is ~10x slower
    (software DGE) and carries only tiny consts.
  - Projections chase the arriving range blocks; attention starts while
    later ranges are still in flight.  Dummy matmuls bridge every DMA stall
    so the PE HAM clock-gate stays at 2.4 GHz.
  - MM1 row-tiled (contraction DL=64): chunk pairs run concurrently on the
    two 64-row halves of the PE array.  kT has a parity layout (even
    kv-ranges' k in partitions 0:64, odd in 64:128) produced with
    per-range-swapped [k|v]/[v|k] projection weights so no cross-partition
    moves are needed; q is duplicated into both halves for free by
    duplicating Wd's q columns (M=64 -> M=128 projection).
  - Postponed normalization: out = (ctxU @ [Wu; bu*Z]) * (1/Z) per query.
    MM2's stationary is [v | ones], so ctx rows 0:64 are unnormalized ctx
    and row 64 is Z; the up-projection contracts over 65 rows (row 64 = bu)
    and the PSUM evacuation is a bias-free per-partition scale by 1/Z that
    either the vector OR scalar engine can apply (tail splits across both).
    ctxU flows bf16 (values span e^+-55: overflows fp16, not bf16).
  - Z is transposed to query-major via 8 tiny f32r PE transposes per pass +
    one reciprocal; scalar engine otherwise does exp ONLY (34 [128,1024]
    ACTIVATEs ~ 38us is the pacing floor).
"""

import sys

if "/opt/trn_rl_repo" not in sys.path:
    sys.path.insert(0, "/opt/trn_rl_repo")

import numpy as np

from concourse import bacc, tile
from concourse import mybir
from concourse.masks import make_identity

F32 = mybir.dt.float32
F32R = mybir.dt.float32r
BF16 = mybir.dt.bfloat16
F16 = mybir.dt.float16

B, S, D, DL = 4, 4096, 1024, 64
N_CORES = 8
S_LOC = S // 2          # 2048 query rows per core
SR = 512
JC = 128                # key chunk
NJK = 17                # compacted key chunks
K_CAP = NJK * JC        # 2176 >= max unmasked keys per batch
QH = 1024               # one attention pass = 1024 query columns
SCALE = 1.25            # 1/sqrt(64)/0.1
LOGIT_SHIFT = -40.0
MASKED_BIAS = -1e30

# kv ranges: (col0, width, parity).  Even ranges project with [k|v] weights
# (k -> psum rows 0:64), odd with [v|k] (k -> rows 64:128), so the k-half
# always evacuates same-partition into its kT2 half.
KV_RANGES = [(0, 512, 0), (512, 512, 1), (1024, 512, 0), (1536, 512, 1),
             (2048, 128, 0)]
# chunk -> kT2 half/block: top (rows 0:64) = ranges 0,2,4; bottom = 1,3
TOP_CHUNKS = [0, 1, 2, 3, 8, 9, 10, 11, 16]
BOT_CHUNKS = [4, 5, 6, 7, 12, 13, 14, 15]
# pass A: step order follows DMA arrival (range 0 chunks first); pass B has
# all data resident -> strict top/bot pairs for full MM1 concurrency
STEPS_A = [(0, 1), (2, 3), (4, 5), (6, 7), (8, 9), (10, 11), (12, 13),
           (14, 15), (16, None)]
STEPS_B = [(0, 4), (1, 5), (2, 6), (3, 7), (8, 12), (9, 13), (10, 14),
           (11, 15), (16, None)]
SOLO = 16

_CACHE = {}


def _chunk_block(c):
    """kT2 (half, block) for chunk c."""
    if c in TOP_CHUNKS:
        return 0, TOP_CHUNKS.index(c)
    return 1, BOT_CHUNKS.index(c)


def build_graph():
    nc = bacc.Bacc("TRN2", target_bir_lowering=False, debug=False,
                   num_devices=N_CORES)

    # Wd packed as: cols 0:2048 = per-chunk [q|q|k|v] (256 each, the only
    # part needed before attention starts), cols 2048:3072 = per-chunk [v|k]
    wd_d = nc.dram_tensor("Wd2", [128, 3072], F16, kind="ExternalInput").ap()
    wu_d = nc.dram_tensor("Wu2", [65, D], BF16, kind="ExternalInput").ap()
    bdq_d = nc.dram_tensor("bd_q2", [128, 1], F32, kind="ExternalInput").ap()
    bdkv_d = nc.dram_tensor("bd_kv2", [128, 2], F32, kind="ExternalInput").ap()
    mb_d = nc.dram_tensor("maskbias", [128, NJK], F32, kind="ExternalInput").ap()
    out_d = nc.dram_tensor("out", [S_LOC, D], F16, kind="ExternalOutput").ap()

    xk_g, xq_g = {}, {}
    for lo in (0, 4):
        h = "lo" if lo == 0 else "hi"
        xk_g[(lo, 0)] = nc.dram_tensor(
            f"xk_a_{h}", [128, 4 * 512], F16, kind="ExternalInput").ap()
        xk_g[(lo, 512)] = nc.dram_tensor(
            f"xk_b_{h}", [128, 4 * 512], F16, kind="ExternalInput").ap()
        xk_g[(lo, 1024)] = nc.dram_tensor(
            f"xk_c2_{h}", [128, 4 * 512], F16, kind="ExternalInput").ap()
        xk_g[(lo, 1536)] = nc.dram_tensor(
            f"xk_c34_{h}", [128, 4 * (K_CAP - 1536)], F16,
            kind="ExternalInput").ap()
        for r in range(4):
            xq_g[(lo, r)] = nc.dram_tensor(
                f"xq_r{r}_{h}", [128, 4 * 512], F16,
                kind="ExternalInput").ap()

    def xk_col(k, c):
        # flat xk_sb column for slab k, key-col c (group-blocked layout)
        half, kl = k // 4, k % 4
        if c < 512:
            return half * 2048 + kl * 512 + c
        if c < 1024:
            return 4096 + half * 2048 + kl * 512 + (c - 512)
        if c < 1536:
            return 8192 + half * 2048 + kl * 512 + (c - 1024)
        return 12288 + half * 2560 + kl * 640 + (c - 1536)

    def xq_col(k, c):
        half, kl = k // 4, k % 4
        return (c // 512) * 4096 + half * 2048 + kl * 512 + (c % 512)

    with tile.TileContext(nc) as tc, nc.allow_low_precision(
            reason="bf16/f16 tiles feed full-rate PE matmuls; ~10-bit "
                   "mantissas are far inside the 2e-2 error budget"):
        with (
            tc.tile_pool(name="consts", bufs=1) as consts,
            tc.tile_pool(name="acts", bufs=1) as acts,
            tc.tile_pool(name="ep", bufs=5) as ep,
            tc.tile_pool(name="ob", bufs=4) as ob,
            tc.tile_pool(name="PL", bufs=2, space="PSUM") as PL,
            tc.tile_pool(name="PC", bufs=1, space="PSUM") as PC,
            tc.tile_pool(name="PP", bufs=1, space="PSUM") as PP,
            tc.tile_pool(name="PT", bufs=1, space="PSUM") as PT,
        ):
            # ---- warm-up seeds, all on the scalar engine (its preamble is
            # the shortest) so PE dummies + ACT table load start ~1us ------
            seed = consts.tile([128, SR], F16)
            nc.scalar.memzero(seed[:])
            actwarm = consts.tile([128, 32], F32)

            # ---- small consts ---------------------------------------------
            identf = consts.tile([128, 64], F32)
            nc.vector.memset(identf[:], 0.0)
            make_identity(nc, identf[0:64, :], nomemset=True)
            make_identity(nc, identf[64:128, :], nomemset=True)
            ident = consts.tile([128, 64], F32R)
            nc.vector.tensor_copy(ident[:], identf[:])
            idzf = consts.tile([128, 4], F32)
            nc.vector.memset(idzf[:], 0.0)
            make_identity(nc, idzf[64:66, 0:2], nomemset=True)
            idz = consts.tile([128, 4], F32R)
            nc.vector.tensor_copy(idz[:], idzf[:])

            # ---- DMA'd consts (gpsimd slow queue: tiny / late-needed) -----
            bdq_s = consts.tile([128, 1], F32)
            nc.gpsimd.dma_start(out=bdq_s[:], in_=bdq_d[:])
            bdkv_s = consts.tile([128, 2], F32)
            nc.gpsimd.dma_start(out=bdkv_s[:], in_=bdkv_d[:])
            mb_s = consts.tile([128, NJK], F32)
            nc.gpsimd.dma_start(out=mb_s[:], in_=mb_d[:])
            wu_s = consts.tile([65, D], BF16)
            nc.gpsimd.dma_start(out=wu_s[:], in_=wu_d[:])
            wd_s = consts.tile([128, 3072], F16)

            # ---- activation tiles (x slabs group-blocked, see xk_col) -----
            xq_sb = acts.tile([128, 8 * S_LOC], F16)
            xk_sb = acts.tile([128, 8 * K_CAP], F16)
            qT2 = acts.tile([128, S_LOC], F16)       # q in both halves
            kT2 = acts.tile([128, 9 * JC], F16)      # parity layout
            vTb = acts.tile([128, K_CAP], F32R)      # v staging (half by rng)
            v_aug = acts.tile([128, NJK * 65], BF16)  # [v(64)|ones] per chunk
            nc.vector.memset(v_aug[:], 1.0)
            ctxu = acts.tile([65, S_LOC], BF16)      # rows 0:64 ctx, 64 = Z
            zr = acts.tile([128, S_LOC], F32R)       # Z row staging (row 64)
            rzbc = acts.tile([128, 32], F32)         # 1/Z query-major, 2*st

            # ---- input DMAs: contiguous blocks, two fast queues -----------
            for lo, eng in ((0, nc.sync), (4, nc.scalar)):
                eng.dma_start(out=wd_s[:, lo * 256:(lo + 4) * 256],
                              in_=wd_d[:, lo * 256:(lo + 4) * 256])
                c = xk_col(lo, 0)
                eng.dma_start(out=xk_sb[:, c:c + 2048], in_=xk_g[(lo, 0)][:])
                c = xq_col(lo, 0)
                eng.dma_start(out=xq_sb[:, c:c + 2048], in_=xq_g[(lo, 0)][:])
                c = xq_col(lo, 512)
                eng.dma_start(out=xq_sb[:, c:c + 2048], in_=xq_g[(lo, 1)][:])
                # odd-range [v|k] weight block
                eng.dma_start(out=wd_s[:, 2048 + lo * 128:2048 + (lo + 4) * 128],
                              in_=wd_d[:, 2048 + lo * 128:2048 + (lo + 4) * 128])
                c = xk_col(lo, 512)
                eng.dma_start(out=xk_sb[:, c:c + 2048], in_=xk_g[(lo, 512)][:])
            for c, g in ((xk_col(0, 1024), xk_g[(0, 1024)]),
                         (xk_col(0, 1536), xk_g[(0, 1536)]),
                         (xq_col(0, 1024), xq_g[(0, 2)]),
                         (xq_col(0, 1536), xq_g[(0, 3)])):
                w = g.shape[1]
                nc.sync.dma_start(out=xk_sb[:, c:c + w] if g in (
                    xk_g[(0, 1024)], xk_g[(0, 1536)]) else
                    xq_sb[:, c:c + w], in_=g[:])

            def late_hi(which):
                # staged scalar triggers: emitted mid-stream once their
                # DMA semaphores are free, so they never block the exps
                if which == 0:
                    c = xk_col(4, 1024)
                    nc.scalar.dma_start(out=xk_sb[:, c:c + 2048],
                                        in_=xk_g[(4, 1024)][:])
                elif which == 1:
                    c = xk_col(4, 1536)
                    nc.scalar.dma_start(out=xk_sb[:, c:c + 2560],
                                        in_=xk_g[(4, 1536)][:])
                elif which == 2:
                    c = xq_col(4, 1024)
                    nc.scalar.dma_start(out=xq_sb[:, c:c + 2048],
                                        in_=xq_g[(4, 2)][:])
                else:
                    c = xq_col(4, 1536)
                    nc.scalar.dma_start(out=xq_sb[:, c:c + 2048],
                                        in_=xq_g[(4, 3)][:])

            # exp ACT table preload (~2.7us) now that scalar's DMA
            # triggers are all enqueued
            nc.scalar.activation(actwarm[:], seed[:, 0:32],
                                 mybir.ActivationFunctionType.Exp)

            # ---- helpers --------------------------------------------------
            ndum = [0]

            def warm(n):
                for _ in range(n):
                    dmy = PL.tile([128, QH], F32, tag="L",
                                  name=f"dmy{ndum[0]}")
                    c0 = (ndum[0] % 2) * SR
                    ndum[0] += 1
                    nc.tensor.matmul(dmy[:, c0:c0 + SR], seed[:, 0:128],
                                     seed[:], start=True, stop=True)

            _ps = {}

            def q_range(r, wm=0, part=None):
                if part != 1:
                    _ps[("q", r)] = PP.tile([128, SR], F32, tag="p",
                                            name=f"psq{r}")
                ps_q = _ps[("q", r)]
                ks = range(8) if part is None else (
                    range(4) if part == 0 else range(4, 8))
                for k in ks:
                    if k == 4 and wm:
                        warm(wm)
                    c = xq_col(k, r * SR)
                    nc.tensor.matmul(
                        ps_q[:], wd_s[:, k * 256:k * 256 + 128],
                        xq_sb[:, c:c + SR],
                        start=(k == 0), stop=(k == 7))
                if part != 0:
                    nc.vector.tensor_scalar_add(
                        qT2[:, r * SR:(r + 1) * SR], ps_q[:], bdq_s[:, 0:1])

            def kv_range(ri, wm=0, part=None):
                c0, w, par = KV_RANGES[ri]
                if part != 1:
                    _ps[("kv", ri)] = PP.tile([128, SR], F32, tag="p",
                                              name=f"pskv{ri}")
                ps_kv = _ps[("kv", ri)]
                ks = range(8) if part is None else (
                    range(4) if part == 0 else range(4, 8))
                for k in ks:
                    if k == 4 and wm:
                        warm(wm)
                    if par == 0:
                        lhsT = wd_s[:, k * 256 + 128:k * 256 + 256]
                    else:
                        lhsT = wd_s[:, 2048 + k * 128:2048 + (k + 1) * 128]
                    c = xk_col(k, c0)
                    nc.tensor.matmul(
                        ps_kv[:, 0:w], lhsT, xk_sb[:, c:c + w],
                        start=(k == 0), stop=(k == 7))
                if part == 0:
                    return
                half, blk0 = _chunk_block(c0 // JC)
                kh = slice(0, 64) if half == 0 else slice(64, 128)
                vh = slice(64, 128) if half == 0 else slice(0, 64)
                nc.vector.tensor_scalar_add(
                    kT2[kh, blk0 * JC:blk0 * JC + w], ps_kv[kh, 0:w],
                    bdkv_s[kh, par:par + 1])
                nc.vector.tensor_scalar_add(
                    vTb[vh, c0:c0 + w], ps_kv[vh, 0:w],
                    bdkv_s[vh, par:par + 1])

            def v_trans(ri):
                c0, w, par = KV_RANGES[ri]
                vh = slice(64, 128) if par == 0 else slice(0, 64)
                idh = ident[64:128, :] if par == 0 else ident[0:64, :]
                nch = w // JC
                vt_ps = PT.tile([128, 256], F32R, tag="t", name=f"vt{ri}")
                for j in range(nch):
                    c = c0 // JC + j
                    nc.tensor.transpose(
                        vt_ps[:, j * 64:(j + 1) * 64],
                        vTb[vh, c * JC:(c + 1) * JC], idh)
                for j in range(nch):
                    c = c0 // JC + j
                    nc.vector.tensor_copy(v_aug[:, c * 65:c * 65 + 64],
                                          vt_ps[:, j * 64:(j + 1) * 64])

            # ================ main software pipeline =======================
            exs = {}
            ctx_tiles = {}
            nmm2 = [0]

            def mm1_exp(pas, ce, co):
                q0 = pas * QH
                lgs = []
                for c in (ce, co):
                    if c is None:
                        continue
                    half, blk = _chunk_block(c)
                    hs = slice(0, 64) if half == 0 else slice(64, 128)
                    lg = PL.tile([128, QH], F32, tag="L",
                                 name=f"lg{pas}_{c}")
                    for s2 in range(2):
                        nc.tensor.matmul(
                            lg[:, s2 * SR:(s2 + 1) * SR],
                            kT2[hs, blk * JC:(blk + 1) * JC],
                            qT2[hs, q0 + s2 * SR:q0 + (s2 + 1) * SR],
                            start=True, stop=True)
                    lgs.append((c, lg))
                for c, lg in lgs:
                    ex = ep.tile([128, QH], BF16, tag="e", name=f"ex{pas}_{c}")
                    nc.scalar.activation(ex[:], lg[:],
                                         mybir.ActivationFunctionType.Exp,
                                         bias=mb_s[:, c:c + 1], scale=SCALE)
                    exs[c] = ex

            def mm2(pas, c):
                ctx_ps = ctx_tiles[pas]
                i = nmm2[0]
                nmm2[0] += 1
                first = (i % NJK == 0)
                last = (i % NJK == NJK - 1)
                for s2 in range(2):
                    nc.tensor.matmul(
                        ctx_ps[:, s2 * SR:(s2 + 1) * SR],
                        v_aug[:, c * 65:(c + 1) * 65],
                        exs[c][:, s2 * SR:(s2 + 1) * SR],
                        start=first, stop=last)

            def ctx_evac(pas):
                q0 = pas * QH
                ctx_ps = ctx_tiles[pas]
                nc.vector.tensor_copy(zr[64:65, q0:q0 + QH], ctx_ps[64:65, :])
                nc.vector.tensor_copy(ctxu[:, q0:q0 + QH], ctx_ps[0:65, :])

            def z_recip(pas):
                # transpose Z [1,1024] -> query-major via 8 tiny f32r PE
                # transposes (K=2: row 65 is a discarded garbage column to
                # satisfy the fp32r ISA restriction), then one reciprocal
                q0 = pas * QH
                zt_ps = PT.tile([128, 16], F32R, tag="t", name=f"zt{pas}")
                for st in range(8):
                    nc.tensor.transpose(
                        zt_ps[:, 2 * st:2 * st + 2],
                        zr[64:66, q0 + st * JC:q0 + (st + 1) * JC],
                        idz[64:66, 0:2])
                nc.vector.reciprocal(rzbc[:, pas * 16:pas * 16 + 16],
                                     zt_ps[:, 0:16])

            def up_tile(st, tail=None):
                # out rows st*128:(st+1)*128 = (ctxu_st @ [Wu; bu]) * 1/Z_q
                osb = ob.tile([128, D], F16, tag="o", name=f"osb{st}")
                up = None
                if tail == "PL":
                    up = PL.tile([128, QH], F32, tag="L", name=f"upt{st}")
                elif tail == "PC":
                    up = PC.tile([128, QH], F32, tag="c", name=f"upc{st}")
                if up is not None:
                    ups = [up[:, 0:SR], up[:, SR:QH]]
                else:
                    ups = [PP.tile([128, SR], F32, tag="p", name=f"up{st}a"),
                           PT.tile([128, SR], F32, tag="t", name=f"up{st}b")]
                for s2 in range(2):
                    nc.tensor.matmul(
                        ups[s2], ctxu[:, st * JC:(st + 1) * JC],
                        wu_s[:, s2 * SR:(s2 + 1) * SR],
                        start=True, stop=True)
                if up is not None:
                    # one [128,1024] evac, engines alternating by st
                    if st % 2 == 0:
                        nc.scalar.mul(osb[:], up[:],
                                      rzbc[:, 2 * st:2 * st + 1])
                    else:
                        nc.vector.tensor_scalar_mul(
                            osb[:], up[:], rzbc[:, 2 * st:2 * st + 1])
                elif tail == "HT":
                    # tail half-tiles: split the two evacs across engines
                    nc.scalar.mul(osb[:, 0:SR], ups[0],
                                  rzbc[:, 2 * st:2 * st + 1])
                    nc.vector.tensor_scalar_mul(
                        osb[:, SR:QH], ups[1], rzbc[:, 2 * st:2 * st + 1])
                else:
                    for s2 in range(2):
                        nc.vector.tensor_scalar_mul(
                            osb[:, s2 * SR:(s2 + 1) * SR], ups[s2],
                            rzbc[:, 2 * st:2 * st + 1])
                eng = nc.sync if (st < 8 or st % 2 == 0) else nc.scalar
                eng.dma_start(out=out_d[st * JC:(st + 1) * JC, :], in_=osb[:])

            # ---- prologue: ranges chase the DMAs, dummies bridge stalls ---
            warm(8)
            kv_range(0, wm=1)
            warm(1)
            q_range(0, wm=1)
            # step (0,0) unrolled: exp halves fire as soon as their q
            # columns are projected (c0 s2=0 needs only q_range(0))
            lg0 = PL.tile([128, QH], F32, tag="L", name="lg0_0")
            ex0 = ep.tile([128, QH], BF16, tag="e", name="ex0_0")
            nc.tensor.matmul(lg0[:, 0:SR], kT2[0:64, 0:JC], qT2[0:64, 0:SR],
                             start=True, stop=True)
            nc.scalar.activation(ex0[:, 0:SR], lg0[:, 0:SR],
                                 mybir.ActivationFunctionType.Exp,
                                 bias=mb_s[:, 0:1], scale=SCALE)
            q_range(1)
            nc.tensor.matmul(lg0[:, SR:QH], kT2[0:64, 0:JC], qT2[0:64, SR:QH],
                             start=True, stop=True)
            nc.scalar.activation(ex0[:, SR:QH], lg0[:, SR:QH],
                                 mybir.ActivationFunctionType.Exp,
                                 bias=mb_s[:, 0:1], scale=SCALE)
            exs[0] = ex0
            lg1 = PL.tile([128, QH], F32, tag="L", name="lg0_1")
            ex1 = ep.tile([128, QH], BF16, tag="e", name="ex0_1")
            for s2 in range(2):
                nc.tensor.matmul(lg1[:, s2 * SR:(s2 + 1) * SR],
                                 kT2[0:64, JC:2 * JC],
                                 qT2[0:64, s2 * SR:(s2 + 1) * SR],
                                 start=True, stop=True)
            nc.scalar.activation(ex1[:], lg1[:],
                                 mybir.ActivationFunctionType.Exp,
                                 bias=mb_s[:, 1:2], scale=SCALE)
            exs[1] = ex1
            v_trans(0)
            kv_range(1, part=0)

            fillers = {
                (0, 1): [lambda: kv_range(1, part=1), lambda: v_trans(1)],
                (0, 2): [lambda: kv_range(2, part=0)],
                (0, 3): [lambda: kv_range(2, part=1), lambda: v_trans(2)],
                (0, 4): [lambda: kv_range(3, part=0)],
                (0, 5): [lambda: kv_range(3, part=1), lambda: v_trans(3)],
                (0, 6): [lambda: kv_range(4), lambda: v_trans(4),
                         lambda: q_range(2, part=0)],
                (0, 7): [lambda: q_range(2, part=1),
                         lambda: q_range(3, part=0)],
                (0, 8): [lambda: q_range(3, part=1)],
                (1, 1): [lambda: up_tile(0)],
                (1, 2): [lambda: up_tile(1)],
                (1, 3): [lambda: up_tile(2)],
                (1, 4): [lambda: up_tile(3)],
                (1, 5): [lambda: up_tile(4)],
                (1, 6): [lambda: up_tile(5)],
                (1, 7): [lambda: up_tile(6)],
                (1, 8): [lambda: up_tile(7)],
            }

            for pas in range(2):
                steps = STEPS_A if pas == 0 else STEPS_B
                ctx_tiles[pas] = PC.tile([65, QH], F32, tag="c",
                                         name=f"ctx{pas}")
                for si, (ce, co) in enumerate(steps):
                    if si > 0:
                        pe, po = steps[si - 1]
                        mm2(pas, pe)
                        if po is not None:
                            mm2(pas, po)
                    elif pas == 1:
                        mm2(0, SOLO)
                        ctx_evac(0)
                    if pas == 0 and si == 0:
                        continue   # unrolled in the prologue
                    mm1_exp(pas, ce, co)
                    if pas == 0 and 1 <= si <= 4:
                        late_hi(si - 1)
                    if pas == 1 and si == 1:
                        z_recip(0)
                    for f in fillers.get((pas, si), []):
                        f()
                if pas == 1:
                    mm2(1, SOLO)
            q1t = 1 * QH
            nc.vector.tensor_copy(zr[64:65, q1t:q1t + QH],
                                  ctx_tiles[1][64:65, :])
            nc.vector.tensor_copy(ctxu[:, q1t:q1t + QH],
                                  ctx_tiles[1][0:65, :])
            z_recip(1)
            pools = ["PL", "PC", "HT"]
            for i, st in enumerate(range(8, 16)):
                up_tile(st, tail=pools[i % 3])

    nc.compile()
    return nc


def get_graph():
    if "graph" not in _CACHE:
        _CACHE["graph"] = build_graph()
    return _CACHE["graph"]


def make_in_maps(x, attention_mask, Wd, bd, Wu, bu):
    # wd2: cols 0:2048 per-chunk [q|q|k|v], cols 2048:3072 per-chunk [v|k]
    wd2 = np.empty((128, 3072), np.float16)
    for k in range(8):
        blk = Wd[k * 128:(k + 1) * 128, :].astype(np.float16)
        q_, k_, v_ = blk[:, 0:64], blk[:, 64:128], blk[:, 128:192]
        wd2[:, k * 256:(k + 1) * 256] = np.concatenate([q_, q_, k_, v_], 1)
        wd2[:, 2048 + k * 128:2048 + (k + 1) * 128] = np.concatenate(
            [v_, k_], 1)
    bf16 = mybir.dt.np(mybir.dt.bfloat16)
    wu2 = np.ascontiguousarray(np.concatenate(
        [Wu, bu.reshape(1, D)], axis=0).astype(bf16))
    bdq2 = np.concatenate([bd[0:64], bd[0:64]]).reshape(128, 1).astype(np.float32)
    bdkv2 = np.stack([
        np.concatenate([bd[64:128], bd[128:192]]),
        np.concatenate([bd[128:192], bd[64:128]]),
    ], axis=1).astype(np.float32)
    per_batch = []
    for b in range(B):
        idx = np.nonzero(attention_mask[b])[0]
        n = len(idx)
        assert n <= K_CAP, f"unmasked key count {n} exceeds K_CAP={K_CAP}"
        idxp = np.concatenate([idx, np.zeros(K_CAP - n, np.int64)])
        xkT = x[b][idxp].T.astype(np.float16).reshape(
            8, 128, K_CAP).transpose(1, 0, 2)   # [128, slab, col]
        mb = np.full(K_CAP, MASKED_BIAS, np.float32)
        mb[:n] = LOGIT_SHIFT
        per_batch.append((xkT, np.ascontiguousarray(mb.reshape(NJK, 128).T)))
    in_maps = []
    for c in range(N_CORES):
        b, h = c // 2, c % 2
        xkT, mb = per_batch[b]
        xT = x[b, h * S_LOC:(h + 1) * S_LOC].T.astype(np.float16).reshape(
            8, 128, S_LOC).transpose(1, 0, 2)
        m = {
            "Wd2": wd2,
            "Wu2": wu2,
            "bd_q2": bdq2,
            "bd_kv2": bdkv2,
            "maskbias": mb,
        }
        for lo in (0, 4):
            hh = "lo" if lo == 0 else "hi"
            sl = slice(lo, lo + 4)
            m[f"xk_a_{hh}"] = np.ascontiguousarray(
                xkT[:, sl, 0:512]).reshape(128, -1)
            m[f"xk_b_{hh}"] = np.ascontiguousarray(
                xkT[:, sl, 512:1024]).reshape(128, -1)
            m[f"xk_c2_{hh}"] = np.ascontiguousarray(
                xkT[:, sl, 1024:1536]).reshape(128, -1)
            m[f"xk_c34_{hh}"] = np.ascontiguousarray(
                xkT[:, sl, 1536:K_CAP]).reshape(128, -1)
            for r in range(4):
                m[f"xq_r{r}_{hh}"] = np.ascontiguousarray(
                    xT[:, sl, r * 512:(r + 1) * 512]).reshape(128, -1)
        in_maps.append(m)
    return in_maps


def kernel(x, attention_mask, Wd, bd, Wu, bu):
    from concourse import bass_utils

    x = np.asarray(x, dtype=np.float32)
    attention_mask = np.asarray(attention_mask)
    Wd = np.asarray(Wd, dtype=np.float32)
    bd = np.asarray(bd, dtype=np.float32)
    Wu = np.asarray(Wu, dtype=np.float32)
    bu = np.asarray(bu, dtype=np.float32)

    nc = get_graph()
    in_maps = make_in_maps(x, attention_mask, Wd, bd, Wu, bu)
    res = bass_utils.run_bass_kernel_spmd(nc, in_maps, list(range(N_CORES)))
    out = np.empty((B, S, D), dtype=np.float32)
    for c in range(N_CORES):
        b, h = c // 2, c % 2
        out[b, h * S_LOC:(h + 1) * S_LOC, :] = \
            res.results[c]["out"].astype(np.float32)
    return out


# revision 64
# speedup vs baseline: 1.0312x; 1.0312x over previous
"""Distributed Trainium2 (Bass/Tile) kernel for single-head latent attention.

Reference computation (B=4, S=4096, D=1024, DL=64):
    qkv = x @ Wd + bd; q,k,v = split(qkv)
    logits = (q @ k^T) / sqrt(DL) / TEMP, key-masked
    out = softmax(logits) @ v @ Wu + bu

Sharding: data-parallel over (batch, seq-half) -> 8 shards of 2048 query
rows; each core recomputes K/V for its batch's compacted keys (no
collectives).

Design notes (vs the 124us baseline):
  - Host-side mask compaction (only ~2048 unmasked keys kept, cap 2176).
  - Inputs are pre-split host-side into C-contiguous per-transfer blocks and
    the SBUF side is laid out identically, so both ends of every DMA are
    4KB+/partition contiguous -> big packets -> full HW-queue bandwidth.
    Only sync + scalar have hardware DGE queues; gpsimd's is ~10x slower
    (software DGE) and carries only tiny consts.
  - Projections chase the arriving range blocks; attention starts while
    later ranges are still in flight.  Dummy matmuls bridge every DMA stall
    so the PE HAM clock-gate stays at 2.4 GHz.
  - MM1 row-tiled (contraction DL=64): chunk pairs run concurrently on the
    two 64-row halves of the PE array.  kT has a parity layout (even
    kv-ranges' k in partitions 0:64, odd in 64:128) produced with
    per-range-swapped [k|v]/[v|k] projection weights so no cross-partition
    moves are needed; q is duplicated into both halves for free by
    duplicating Wd's q columns (M=64 -> M=128 projection).
  - Postponed normalization: out = (ctxU @ [Wu; bu*Z]) * (1/Z) per query.
    MM2's stationary is [v | ones], so ctx rows 0:64 are unnormalized ctx
    and row 64 is Z; the up-projection contracts over 65 rows (row 64 = bu)
    and the PSUM evacuation is a bias-free per-partition scale by 1/Z that
    either the vector OR scalar engine can apply (tail splits across both).
    ctxU flows bf16 (values span e^+-55: overflows fp16, not bf16).
  - Z is transposed to query-major via 8 tiny f32r PE transposes per pass +
    one reciprocal; scalar engine otherwise does exp ONLY (34 [128,1024]
    ACTIVATEs ~ 38us is the pacing floor).
"""

import sys

if "/opt/trn_rl_repo" not in sys.path:
    sys.path.insert(0, "/opt/trn_rl_repo")

import numpy as np

from concourse import bacc, tile
from concourse import mybir
from concourse.masks import make_identity

F32 = mybir.dt.float32
F32R = mybir.dt.float32r
BF16 = mybir.dt.bfloat16
F16 = mybir.dt.float16

B, S, D, DL = 4, 4096, 1024, 64
N_CORES = 8
S_LOC = S // 2          # 2048 query rows per core
SR = 512
JC = 128                # key chunk
NJK = 17                # compacted key chunks
K_CAP = NJK * JC        # 2176 >= max unmasked keys per batch
QH = 1024               # one attention pass = 1024 query columns
SCALE = 1.25            # 1/sqrt(64)/0.1
LOGIT_SHIFT = -40.0
MASKED_BIAS = -1e30

# kv ranges: (col0, width, parity).  Even ranges project with [k|v] weights
# (k -> psum rows 0:64), odd with [v|k] (k -> rows 64:128), so the k-half
# always evacuates same-partition into its kT2 half.
KV_RANGES = [(0, 512, 0), (512, 512, 1), (1024, 512, 0), (1536, 512, 1),
             (2048, 128, 0)]
# chunk -> kT2 half/block: top (rows 0:64) = ranges 0,2,4; bottom = 1,3
TOP_CHUNKS = [0, 1, 2, 3, 8, 9, 10, 11, 16]
BOT_CHUNKS = [4, 5, 6, 7, 12, 13, 14, 15]
# pass A: step order follows DMA arrival (range 0 chunks first); pass B has
# all data resident -> strict top/bot pairs for full MM1 concurrency
STEPS_A = [(0, 1), (2, 3), (4, 5), (6, 7), (8, 9), (10, 11), (12, 13),
           (14, 15), (16, None)]
STEPS_B = [(0, 4), (1, 5), (2, 6), (3, 7), (8, 12), (9, 13), (10, 14),
           (11, 15), (16, None)]
SOLO = 16

_CACHE = {}


def _chunk_block(c):
    """kT2 (half, block) for chunk c."""
    if c in TOP_CHUNKS:
        return 0, TOP_CHUNKS.index(c)
    return 1, BOT_CHUNKS.index(c)


def build_graph():
    nc = bacc.Bacc("TRN2", target_bir_lowering=False, debug=False,
                   num_devices=N_CORES)

    # Wd packed as: cols 0:2048 = per-chunk [q|q|k|v] (256 each, the only
    # part needed before attention starts), cols 2048:3072 = per-chunk [v|k]
    wd_d = nc.dram_tensor("Wd2", [128, 3072], F16, kind="ExternalInput").ap()
    wu_d = nc.dram_tensor("Wu2", [65, D], BF16, kind="ExternalInput").ap()
    bdq_d = nc.dram_tensor("bd_q2", [128, 1], F32, kind="ExternalInput").ap()
    bdkv_d = nc.dram_tensor("bd_kv2", [128, 2], F32, kind="ExternalInput").ap()
    mb_d = nc.dram_tensor("maskbias", [128, NJK], F32, kind="ExternalInput").ap()
    out_d = nc.dram_tensor("out", [S_LOC, D], F16, kind="ExternalOutput").ap()

    xk_g, xq_g = {}, {}
    for lo in (0, 4):
        h = "lo" if lo == 0 else "hi"
        xk_g[(lo, 0)] = nc.dram_tensor(
            f"xk_a_{h}", [128, 4 * 512], F16, kind="ExternalInput").ap()
        xk_g[(lo, 512)] = nc.dram_tensor(
            f"xk_b_{h}", [128, 4 * 512], F16, kind="ExternalInput").ap()
        xk_g[(lo, 1024)] = nc.dram_tensor(
            f"xk_c2_{h}", [128, 4 * 512], F16, kind="ExternalInput").ap()
        xk_g[(lo, 1536)] = nc.dram_tensor(
            f"xk_c34_{h}", [128, 4 * (K_CAP - 1536)], F16,
            kind="ExternalInput").ap()
        for r in range(4):
            xq_g[(lo, r)] = nc.dram_tensor(
                f"xq_r{r}_{h}", [128, 4 * 512], F16,
                kind="ExternalInput").ap()

    def xk_col(k, c):
        # flat xk_sb column for slab k, key-col c (group-blocked layout)
        half, kl = k // 4, k % 4
        if c < 512:
            return half * 2048 + kl * 512 + c
        if c < 1024:
            return 4096 + half * 2048 + kl * 512 + (c - 512)
        if c < 1536:
            return 8192 + half * 2048 + kl * 512 + (c - 1024)
        return 12288 + half * 2560 + kl * 640 + (c - 1536)

    def xq_col(k, c):
        half, kl = k // 4, k % 4
        return (c // 512) * 4096 + half * 2048 + kl * 512 + (c % 512)

    with tile.TileContext(nc) as tc, nc.allow_low_precision(
            reason="bf16/f16 tiles feed full-rate PE matmuls; ~10-bit "
                   "mantissas are far inside the 2e-2 error budget"):
        with (
            tc.tile_pool(name="consts", bufs=1) as consts,
            tc.tile_pool(name="acts", bufs=1) as acts,
            tc.tile_pool(name="ep", bufs=6) as ep,
            tc.tile_pool(name="ob", bufs=4) as ob,
            tc.tile_pool(name="PL", bufs=2, space="PSUM") as PL,
            tc.tile_pool(name="PC", bufs=1, space="PSUM") as PC,
            tc.tile_pool(name="PP", bufs=1, space="PSUM") as PP,
            tc.tile_pool(name="PT", bufs=1, space="PSUM") as PT,
        ):
            # ---- warm-up seeds, all on the scalar engine (its preamble is
            # the shortest) so PE dummies + ACT table load start ~1us ------
            seed = consts.tile([128, SR], F16)
            nc.scalar.memzero(seed[:])
            actwarm = consts.tile([128, 32], F32)

            # ---- small consts ---------------------------------------------
            identf = consts.tile([128, 64], F32)
            nc.vector.memset(identf[:], 0.0)
            make_identity(nc, identf[0:64, :], nomemset=True)
            make_identity(nc, identf[64:128, :], nomemset=True)
            ident = consts.tile([128, 64], F32R)
            nc.vector.tensor_copy(ident[:], identf[:])
            idzf = consts.tile([128, 4], F32)
            nc.vector.memset(idzf[:], 0.0)
            make_identity(nc, idzf[64:66, 0:2], nomemset=True)
            idz = consts.tile([128, 4], F32R)
            nc.vector.tensor_copy(idz[:], idzf[:])

            # ---- DMA'd consts (gpsimd slow queue: tiny / late-needed) -----
            bdq_s = consts.tile([128, 1], F32)
            nc.gpsimd.dma_start(out=bdq_s[:], in_=bdq_d[:])
            bdkv_s = consts.tile([128, 2], F32)
            nc.gpsimd.dma_start(out=bdkv_s[:], in_=bdkv_d[:])
            mb_s = consts.tile([128, NJK], F32)
            nc.gpsimd.dma_start(out=mb_s[:], in_=mb_d[:])
            wu_s = consts.tile([65, D], BF16)
            nc.gpsimd.dma_start(out=wu_s[:], in_=wu_d[:])
            wd_s = consts.tile([128, 3072], F16)

            # ---- activation tiles (x slabs group-blocked, see xk_col) -----
            xq_sb = acts.tile([128, 8 * S_LOC], F16)
            xk_sb = acts.tile([128, 8 * K_CAP], F16)
            qT2 = acts.tile([128, S_LOC], F16)       # q in both halves
            kT2 = acts.tile([128, 9 * JC], F16)      # parity layout
            vTb = acts.tile([128, K_CAP], F32R)      # v staging (half by rng)
            v_aug = acts.tile([128, NJK * 65], BF16)  # [v(64)|ones] per chunk
            nc.vector.memset(v_aug[:], 1.0)
            ctxu = acts.tile([65, S_LOC], BF16)      # rows 0:64 ctx, 64 = Z
            zr = acts.tile([128, S_LOC], F32R)       # Z row staging (row 64)
            rzbc = acts.tile([128, 32], F32)         # 1/Z query-major, 2*st

            # ---- input DMAs: contiguous blocks, two fast queues -----------
            for lo, eng in ((0, nc.sync), (4, nc.scalar)):
                eng.dma_start(out=wd_s[:, lo * 256:(lo + 4) * 256],
                              in_=wd_d[:, lo * 256:(lo + 4) * 256])
                c = xk_col(lo, 0)
                eng.dma_start(out=xk_sb[:, c:c + 2048], in_=xk_g[(lo, 0)][:])
                c = xq_col(lo, 0)
                eng.dma_start(out=xq_sb[:, c:c + 2048], in_=xq_g[(lo, 0)][:])
                c = xq_col(lo, 512)
                eng.dma_start(out=xq_sb[:, c:c + 2048], in_=xq_g[(lo, 1)][:])
                # odd-range [v|k] weight block
                eng.dma_start(out=wd_s[:, 2048 + lo * 128:2048 + (lo + 4) * 128],
                              in_=wd_d[:, 2048 + lo * 128:2048 + (lo + 4) * 128])
                c = xk_col(lo, 512)
                eng.dma_start(out=xk_sb[:, c:c + 2048], in_=xk_g[(lo, 512)][:])
            for c, g in ((xk_col(0, 1024), xk_g[(0, 1024)]),
                         (xk_col(0, 1536), xk_g[(0, 1536)]),
                         (xq_col(0, 1024), xq_g[(0, 2)]),
                         (xq_col(0, 1536), xq_g[(0, 3)])):
                w = g.shape[1]
                nc.sync.dma_start(out=xk_sb[:, c:c + w] if g in (
                    xk_g[(0, 1024)], xk_g[(0, 1536)]) else
                    xq_sb[:, c:c + w], in_=g[:])

            def late_hi(which):
                # staged scalar triggers: emitted mid-stream once their
                # DMA semaphores are free, so they never block the exps
                if which == 0:
                    c = xk_col(4, 1024)
                    nc.scalar.dma_start(out=xk_sb[:, c:c + 2048],
                                        in_=xk_g[(4, 1024)][:])
                elif which == 1:
                    c = xk_col(4, 1536)
                    nc.scalar.dma_start(out=xk_sb[:, c:c + 2560],
                                        in_=xk_g[(4, 1536)][:])
                elif which == 2:
                    c = xq_col(4, 1024)
                    nc.scalar.dma_start(out=xq_sb[:, c:c + 2048],
                                        in_=xq_g[(4, 2)][:])
                else:
                    c = xq_col(4, 1536)
                    nc.scalar.dma_start(out=xq_sb[:, c:c + 2048],
                                        in_=xq_g[(4, 3)][:])

            # exp ACT table preload (~2.7us) now that scalar's DMA
            # triggers are all enqueued
            nc.scalar.activation(actwarm[:], seed[:, 0:32],
                                 mybir.ActivationFunctionType.Exp)

            # ---- helpers --------------------------------------------------
            ndum = [0]

            def warm(n):
                for _ in range(n):
                    dmy = PL.tile([128, QH], F32, tag="L",
                                  name=f"dmy{ndum[0]}")
                    c0 = (ndum[0] % 2) * SR
                    ndum[0] += 1
                    nc.tensor.matmul(dmy[:, c0:c0 + SR], seed[:, 0:128],
                                     seed[:], start=True, stop=True)

            _ps = {}

            def q_range(r, wm=0, part=None):
                if part != 1:
                    _ps[("q", r)] = PP.tile([128, SR], F32, tag="p",
                                            name=f"psq{r}")
                ps_q = _ps[("q", r)]
                ks = range(8) if part is None else (
                    range(4) if part == 0 else range(4, 8))
                for k in ks:
                    if k == 4 and wm:
                        warm(wm)
                    c = xq_col(k, r * SR)
                    nc.tensor.matmul(
                        ps_q[:], wd_s[:, k * 256:k * 256 + 128],
                        xq_sb[:, c:c + SR],
                        start=(k == 0), stop=(k == 7))
                if part != 0:
                    nc.vector.tensor_scalar_add(
                        qT2[:, r * SR:(r + 1) * SR], ps_q[:], bdq_s[:, 0:1])

            def kv_range(ri, wm=0, part=None):
                c0, w, par = KV_RANGES[ri]
                if part != 1:
                    _ps[("kv", ri)] = PP.tile([128, SR], F32, tag="p",
                                              name=f"pskv{ri}")
                ps_kv = _ps[("kv", ri)]
                ks = range(8) if part is None else (
                    range(4) if part == 0 else range(4, 8))
                for k in ks:
                    if k == 4 and wm:
                        warm(wm)
                    if par == 0:
                        lhsT = wd_s[:, k * 256 + 128:k * 256 + 256]
                    else:
                        lhsT = wd_s[:, 2048 + k * 128:2048 + (k + 1) * 128]
                    c = xk_col(k, c0)
                    nc.tensor.matmul(
                        ps_kv[:, 0:w], lhsT, xk_sb[:, c:c + w],
                        start=(k == 0), stop=(k == 7))
                if part == 0:
                    return
                half, blk0 = _chunk_block(c0 // JC)
                kh = slice(0, 64) if half == 0 else slice(64, 128)
                vh = slice(64, 128) if half == 0 else slice(0, 64)
                nc.vector.tensor_scalar_add(
                    kT2[kh, blk0 * JC:blk0 * JC + w], ps_kv[kh, 0:w],
                    bdkv_s[kh, par:par + 1])
                nc.vector.tensor_scalar_add(
                    vTb[vh, c0:c0 + w], ps_kv[vh, 0:w],
                    bdkv_s[vh, par:par + 1])

            def v_trans(ri):
                c0, w, par = KV_RANGES[ri]
                vh = slice(64, 128) if par == 0 else slice(0, 64)
                idh = ident[64:128, :] if par == 0 else ident[0:64, :]
                nch = w // JC
                vt_ps = PT.tile([128, 256], F32R, tag="t", name=f"vt{ri}")
                for j in range(nch):
                    c = c0 // JC + j
                    nc.tensor.transpose(
                        vt_ps[:, j * 64:(j + 1) * 64],
                        vTb[vh, c * JC:(c + 1) * JC], idh)
                for j in range(nch):
                    c = c0 // JC + j
                    nc.vector.tensor_copy(v_aug[:, c * 65:c * 65 + 64],
                                          vt_ps[:, j * 64:(j + 1) * 64])

            # ================ main software pipeline =======================
            exs = {}
            ctx_tiles = {}
            nmm2 = [0]

            def mm1_exp(pas, ce, co):
                q0 = pas * QH
                lgs = []
                for c in (ce, co):
                    if c is None:
                        continue
                    half, blk = _chunk_block(c)
                    hs = slice(0, 64) if half == 0 else slice(64, 128)
                    lg = PL.tile([128, QH], F32, tag="L",
                                 name=f"lg{pas}_{c}")
                    for s2 in range(2):
                        nc.tensor.matmul(
                            lg[:, s2 * SR:(s2 + 1) * SR],
                            kT2[hs, blk * JC:(blk + 1) * JC],
                            qT2[hs, q0 + s2 * SR:q0 + (s2 + 1) * SR],
                            start=True, stop=True)
                    lgs.append((c, lg))
                for c, lg in lgs:
                    ex = ep.tile([128, QH], BF16, tag="e", name=f"ex{pas}_{c}")
                    nc.scalar.activation(ex[:], lg[:],
                                         mybir.ActivationFunctionType.Exp,
                                         bias=mb_s[:, c:c + 1], scale=SCALE)
                    exs[c] = ex

            def mm2(pas, c):
                ctx_ps = ctx_tiles[pas]
                i = nmm2[0]
                nmm2[0] += 1
                first = (i % NJK == 0)
                last = (i % NJK == NJK - 1)
                for s2 in range(2):
                    nc.tensor.matmul(
                        ctx_ps[:, s2 * SR:(s2 + 1) * SR],
                        v_aug[:, c * 65:(c + 1) * 65],
                        exs[c][:, s2 * SR:(s2 + 1) * SR],
                        start=first, stop=last)

            def ctx_evac(pas):
                q0 = pas * QH
                ctx_ps = ctx_tiles[pas]
                nc.vector.tensor_copy(zr[64:65, q0:q0 + QH], ctx_ps[64:65, :])
                nc.vector.tensor_copy(ctxu[:, q0:q0 + QH], ctx_ps[0:65, :])

            def z_recip(pas):
                # transpose Z [1,1024] -> query-major via 8 tiny f32r PE
                # transposes (K=2: row 65 is a discarded garbage column to
                # satisfy the fp32r ISA restriction), then one reciprocal
                q0 = pas * QH
                zt_ps = PT.tile([128, 16], F32R, tag="t", name=f"zt{pas}")
                for st in range(8):
                    nc.tensor.transpose(
                        zt_ps[:, 2 * st:2 * st + 2],
                        zr[64:66, q0 + st * JC:q0 + (st + 1) * JC],
                        idz[64:66, 0:2])
                nc.vector.reciprocal(rzbc[:, pas * 16:pas * 16 + 16],
                                     zt_ps[:, 0:16])

            def up_tile(st, tail=None):
                # out rows st*128:(st+1)*128 = (ctxu_st @ [Wu; bu]) * 1/Z_q
                osb = ob.tile([128, D], F16, tag="o", name=f"osb{st}")
                up = None
                if tail == "PL":
                    up = PL.tile([128, QH], F32, tag="L", name=f"upt{st}")
                elif tail == "PC":
                    up = PC.tile([128, QH], F32, tag="c", name=f"upc{st}")
                if up is not None:
                    ups = [up[:, 0:SR], up[:, SR:QH]]
                else:
                    ups = [PP.tile([128, SR], F32, tag="p", name=f"up{st}a"),
                           PT.tile([128, SR], F32, tag="t", name=f"up{st}b")]
                for s2 in range(2):
                    nc.tensor.matmul(
                        ups[s2], ctxu[:, st * JC:(st + 1) * JC],
                        wu_s[:, s2 * SR:(s2 + 1) * SR],
                        start=True, stop=True)
                if up is not None:
                    # one [128,1024] evac, engines alternating by st
                    if st % 2 == 0:
                        nc.scalar.mul(osb[:], up[:],
                                      rzbc[:, 2 * st:2 * st + 1])
                    else:
                        nc.vector.tensor_scalar_mul(
                            osb[:], up[:], rzbc[:, 2 * st:2 * st + 1])
                elif tail == "HT":
                    # tail half-tiles: split the two evacs across engines
                    nc.scalar.mul(osb[:, 0:SR], ups[0],
                                  rzbc[:, 2 * st:2 * st + 1])
                    nc.vector.tensor_scalar_mul(
                        osb[:, SR:QH], ups[1], rzbc[:, 2 * st:2 * st + 1])
                else:
                    for s2 in range(2):
                        nc.vector.tensor_scalar_mul(
                            osb[:, s2 * SR:(s2 + 1) * SR], ups[s2],
                            rzbc[:, 2 * st:2 * st + 1])
                eng = nc.sync if (st < 8 or st % 2 == 0) else nc.scalar
                eng.dma_start(out=out_d[st * JC:(st + 1) * JC, :], in_=osb[:])

            # ---- prologue: ranges chase the DMAs, dummies bridge stalls ---
            warm(8)
            kv_range(0, wm=1)
            warm(1)
            q_range(0, wm=1)
            # step (0,0) unrolled: exp halves fire as soon as their q
            # columns are projected (c0 s2=0 needs only q_range(0))
            lg0 = PL.tile([128, QH], F32, tag="L", name="lg0_0")
            ex0 = ep.tile([128, QH], BF16, tag="e", name="ex0_0")
            nc.tensor.matmul(lg0[:, 0:SR], kT2[0:64, 0:JC], qT2[0:64, 0:SR],
                             start=True, stop=True)
            nc.scalar.activation(ex0[:, 0:SR], lg0[:, 0:SR],
                                 mybir.ActivationFunctionType.Exp,
                                 bias=mb_s[:, 0:1], scale=SCALE)
            q_range(1)
            nc.tensor.matmul(lg0[:, SR:QH], kT2[0:64, 0:JC], qT2[0:64, SR:QH],
                             start=True, stop=True)
            nc.scalar.activation(ex0[:, SR:QH], lg0[:, SR:QH],
                                 mybir.ActivationFunctionType.Exp,
                                 bias=mb_s[:, 0:1], scale=SCALE)
            exs[0] = ex0
            lg1 = PL.tile([128, QH], F32, tag="L", name="lg0_1")
            ex1 = ep.tile([128, QH], BF16, tag="e", name="ex0_1")
            for s2 in range(2):
                nc.tensor.matmul(lg1[:, s2 * SR:(s2 + 1) * SR],
                                 kT2[0:64, JC:2 * JC],
                                 qT2[0:64, s2 * SR:(s2 + 1) * SR],
                                 start=True, stop=True)
            nc.scalar.activation(ex1[:], lg1[:],
                                 mybir.ActivationFunctionType.Exp,
                                 bias=mb_s[:, 1:2], scale=SCALE)
            exs[1] = ex1
            v_trans(0)
            kv_range(1, part=0)

            fillers = {
                (0, 1): [lambda: kv_range(1, part=1), lambda: v_trans(1)],
                (0, 2): [lambda: kv_range(2, part=0)],
                (0, 3): [lambda: kv_range(2, part=1), lambda: v_trans(2)],
                (0, 4): [lambda: kv_range(3, part=0)],
                (0, 5): [lambda: kv_range(3, part=1), lambda: v_trans(3)],
                (0, 6): [lambda: kv_range(4), lambda: v_trans(4)],
                (0, 7): [lambda: q_range(2)],
                (0, 8): [lambda: q_range(3)],
                (1, 1): [lambda: up_tile(0)],
                (1, 2): [lambda: up_tile(1)],
                (1, 3): [lambda: up_tile(2)],
                (1, 4): [lambda: up_tile(3)],
                (1, 5): [lambda: up_tile(4)],
                (1, 6): [lambda: up_tile(5)],
                (1, 7): [lambda: up_tile(6)],
                (1, 8): [lambda: up_tile(7)],
            }

            for pas in range(2):
                steps = STEPS_A if pas == 0 else STEPS_B
                ctx_tiles[pas] = PC.tile([65, QH], F32, tag="c",
                                         name=f"ctx{pas}")
                for si, (ce, co) in enumerate(steps):
                    if si > 0:
                        pe, po = steps[si - 1]
                        mm2(pas, pe)
                        if po is not None:
                            mm2(pas, po)
                    elif pas == 1:
                        mm2(0, SOLO)
                        ctx_evac(0)
                    if pas == 0 and si == 0:
                        continue   # unrolled in the prologue
                    mm1_exp(pas, ce, co)
                    if pas == 0 and 1 <= si <= 4:
                        late_hi(si - 1)
                    if pas == 1 and si == 1:
                        z_recip(0)
                    for f in fillers.get((pas, si), []):
                        f()
                if pas == 1:
                    mm2(1, SOLO)
            q1t = 1 * QH
            nc.vector.tensor_copy(zr[64:65, q1t:q1t + QH],
                                  ctx_tiles[1][64:65, :])
            nc.vector.tensor_copy(ctxu[:, q1t:q1t + SR],
                                  ctx_tiles[1][0:65, 0:SR])
            nc.vector.tensor_copy(ctxu[:, q1t + SR:q1t + QH],
                                  ctx_tiles[1][0:65, SR:QH])
            z_recip(1)
            pools = ["PL", "PC", "HT"]
            for i, st in enumerate(range(8, 16)):
                up_tile(st, tail=pools[i % 3])

    nc.compile()
    return nc


def get_graph():
    if "graph" not in _CACHE:
        _CACHE["graph"] = build_graph()
    return _CACHE["graph"]


def make_in_maps(x, attention_mask, Wd, bd, Wu, bu):
    # wd2: cols 0:2048 per-chunk [q|q|k|v], cols 2048:3072 per-chunk [v|k]
    wd2 = np.empty((128, 3072), np.float16)
    for k in range(8):
        blk = Wd[k * 128:(k + 1) * 128, :].astype(np.float16)
        q_, k_, v_ = blk[:, 0:64], blk[:, 64:128], blk[:, 128:192]
        wd2[:, k * 256:(k + 1) * 256] = np.concatenate([q_, q_, k_, v_], 1)
        wd2[:, 2048 + k * 128:2048 + (k + 1) * 128] = np.concatenate(
            [v_, k_], 1)
    bf16 = mybir.dt.np(mybir.dt.bfloat16)
    wu2 = np.ascontiguousarray(np.concatenate(
        [Wu, bu.reshape(1, D)], axis=0).astype(bf16))
    bdq2 = np.concatenate([bd[0:64], bd[0:64]]).reshape(128, 1).astype(np.float32)
    bdkv2 = np.stack([
        np.concatenate([bd[64:128], bd[128:192]]),
        np.concatenate([bd[128:192], bd[64:128]]),
    ], axis=1).astype(np.float32)
    per_batch = []
    for b in range(B):
        idx = np.nonzero(attention_mask[b])[0]
        n = len(idx)
        assert n <= K_CAP, f"unmasked key count {n} exceeds K_CAP={K_CAP}"
        idxp = np.concatenate([idx, np.zeros(K_CAP - n, np.int64)])
        xkT = x[b][idxp].T.astype(np.float16).reshape(
            8, 128, K_CAP).transpose(1, 0, 2)   # [128, slab, col]
        mb = np.full(K_CAP, MASKED_BIAS, np.float32)
        mb[:n] = LOGIT_SHIFT
        per_batch.append((xkT, np.ascontiguousarray(mb.reshape(NJK, 128).T)))
    in_maps = []
    for c in range(N_CORES):
        b, h = c // 2, c % 2
        xkT, mb = per_batch[b]
        xT = x[b, h * S_LOC:(h + 1) * S_LOC].T.astype(np.float16).reshape(
            8, 128, S_LOC).transpose(1, 0, 2)
        m = {
            "Wd2": wd2,
            "Wu2": wu2,
            "bd_q2": bdq2,
            "bd_kv2": bdkv2,
            "maskbias": mb,
        }
        for lo in (0, 4):
            hh = "lo" if lo == 0 else "hi"
            sl = slice(lo, lo + 4)
            m[f"xk_a_{hh}"] = np.ascontiguousarray(
                xkT[:, sl, 0:512]).reshape(128, -1)
            m[f"xk_b_{hh}"] = np.ascontiguousarray(
                xkT[:, sl, 512:1024]).reshape(128, -1)
            m[f"xk_c2_{hh}"] = np.ascontiguousarray(
                xkT[:, sl, 1024:1536]).reshape(128, -1)
            m[f"xk_c34_{hh}"] = np.ascontiguousarray(
                xkT[:, sl, 1536:K_CAP]).reshape(128, -1)
            for r in range(4):
                m[f"xq_r{r}_{hh}"] = np.ascontiguousarray(
                    xT[:, sl, r * 512:(r + 1) * 512]).reshape(128, -1)
        in_maps.append(m)
    return in_maps


def kernel(x, attention_mask, Wd, bd, Wu, bu):
    from concourse import bass_utils

    x = np.asarray(x, dtype=np.float32)
    attention_mask = np.asarray(attention_mask)
    Wd = np.asarray(Wd, dtype=np.float32)
    bd = np.asarray(bd, dtype=np.float32)
    Wu = np.asarray(Wu, dtype=np.float32)
    bu = np.asarray(bu, dtype=np.float32)

    nc = get_graph()
    in_maps = make_in_maps(x, attention_mask, Wd, bd, Wu, bu)
    res = bass_utils.run_bass_kernel_spmd(nc, in_maps, list(range(N_CORES)))
    out = np.empty((B, S, D), dtype=np.float32)
    for c in range(N_CORES):
        b, h = c // 2, c % 2
        out[b, h * S_LOC:(h + 1) * S_LOC, :] = \
            res.results[c]["out"].astype(np.float32)
    return out
